# revision 1
# baseline (speedup 1.0000x reference)
"""Trainium2 Bass kernel for dual-stream cross/self attention (nn_Attention).

Reference semantics (per batch b):
  qkv_s = x_s @ Wqkv  -> q_s,k_s,v_s  [H=16 heads, N=577 tokens, d=64]
  stream s output head h attends with q_s and (k_s,v_s) if h<10 else (k_o,v_o)
  out_s = concat_heads @ Wproj + bproj

Sharding: batch (16) data-parallel over 8 cores, 2 batches/core; weights
replicated. Per core, 4 sequences (2 batches x 2 streams) are processed.

Per-core dataflow (all matmuls contract over the SBUF partition dim):
  - host supplies x pre-transposed (xt: [c_in, tok]) and tile-packed weights
  - q,k computed as [c_out, tok] (c-major); v as [tok, c_out] (tok-major),
    with a constant ones-column appended per head so the attention matmul
    also produces the softmax denominator for free
  - scores^T[j,i] = k_head^T q_head, exp via ScalarE (scale fused, no max
    subtraction -- |score*scale| <= ~3.5 for this problem so exp is safe)
  - attn^T[d,i] (+ sumexp row) = [v_head|1]^T @ exp^T, accumulated over j
  - normalize along tokens via reciprocal + DMA partition-broadcast + mult
  - proj: out[tok,c] = attn^T-tiles^T @ Wproj-tiles, bias added from a
    host-replicated [128,1024] bias tile
"""

import numpy as np

import concourse.bass as bass
import concourse.mybir as mybir
import concourse.tile as tile
from concourse.bass_utils import run_bass_kernel_spmd

# ---------------------------------------------------------------------------
# Workaround: this walrus build rejects any instruction carrying >1 sem wait
# ("Too many sync wait commands").  Post-process the scheduled program and
# move excess waits onto single-wait NoOps inserted just before, on the same
# engine (engines execute their stream in order, so this is equivalent).
# ---------------------------------------------------------------------------


def split_excess_waits(nc, max_waits=1):
    cnt = 0
    for f in nc.m.functions:
        for blk in f.blocks:
            insts = blk.instructions
            need = any(
                inst.sync_info is not None
                and len(inst.sync_info.on_wait) > max_waits
                for inst in insts
            )
            if not need:
                continue
            newl = []
            for inst in insts:
                si = inst.sync_info
                if si is not None and len(si.on_wait) > max_waits:
                    waits = list(si.on_wait)
                    for w in waits[max_waits:]:
                        nop = mybir.InstNoOp(
                            name=f"wsplit_{cnt}",
                            engine=inst.engine,
                            ins=[],
                            outs=[],
                            sync_info=mybir.SyncInfo(on_wait=[w], on_update=[]),
                        )
                        cnt += 1
                        newl.append(nop)
                    si.on_wait = waits[:max_waits]
                newl.append(inst)
            blk.instructions = newl
    return cnt

# ---------------------------------------------------------------------------

F32 = mybir.dt.float32

N = 577          # tokens
C = 1024         # model dim
H = 16           # heads
D = 64           # head dim
HS = 10          # first HS heads self-attend, rest cross-attend
KT = 8           # c_in tiles of 128
SCALE = D ** -0.5
NCORES = 8
BL = 2           # local batches per core
NSEQ = 2 * BL    # sequences per core (batch-major, stream-minor)

# token partition tiles (start, len)
TOKT = [(0, 128), (128, 128), (256, 128), (384, 128), (512, 65)]
# token free-dim chunks (start, len): overlap 1 col at 288 so both are 289
# wide and a single ScalarE op can cover both PSUM sub-banks garbage-free
CH = [(0, 289), (288, 289)]

def build_kernel(cdt, reps=1, mode="full"):
    nc = bass.Bass()
    xt = nc.dram_tensor("xt", [NSEQ, C, N], cdt, kind="ExternalInput")
    wqkv = nc.dram_tensor("wqkv", [KT, 24, 128, 128], cdt, kind="ExternalInput")
    wproj = nc.dram_tensor("wproj", [KT, 8, 128, 128], cdt, kind="ExternalInput")
    biasr = nc.dram_tensor("biasr", [128, C], F32, kind="ExternalInput")
    out = nc.dram_tensor("out", [NSEQ, N, C], F32, kind="ExternalOutput")

    import contextlib
    import itertools
    _uid = itertools.count()

    with tile.TileContext(nc) as tc:
        with (
            tc.tile_pool(name="const", bufs=1) as constp,
            tc.tile_pool(name="xa", bufs=4) as xap,       # xt + attnT share
            tc.tile_pool(name="qk", bufs=8) as qkp,       # q,k of 2 batches
            tc.tile_pool(name="vp", bufs=2) as vpp,
            tc.tile_pool(name="ep", bufs=4) as epp,
            tc.tile_pool(name="w1", bufs=16) as w1p,      # streamed qk weights
            tc.tile_pool(name="w8v", bufs=8) as w8vp,     # wv tiles
            tc.tile_pool(name="w8p", bufs=8) as w8pp,     # wproj tiles
            tc.tile_pool(name="rbp", bufs=4) as rbpp,     # recip + broadcast
            tc.tile_pool(name="stg", bufs=2) as stgp,     # odd-head staging
            tc.tile_pool(name="op", bufs=2) as outp,
            tc.tile_pool(name="dr", bufs=4, space="DRAM") as drp,
            tc.tile_pool(name="ps", bufs=2, space="PSUM") as psp,
        ):
            bias_sb = constp.tile([128, C], F32, tag="bias")
            nc.sync.dma_start(out=bias_sb[:], in_=biasr[:])

            state = {}

            def load_batch_inputs(b):
                st = {}
                st["xts"] = []
                for s in range(2):
                    t = xap.tile([128, KT, N], cdt, tag="xa", name=f"xt_{next(_uid)}")
                    nc.sync.dma_start(
                        out=t[:],
                        in_=xt[2 * b + s].rearrange("(kt p) n -> p kt n", p=128),
                    )
                    st["xts"].append(t)
                st["q"] = [
                    qkp.tile([128, 8, N], cdt, tag="qk", name=f"q_{next(_uid)}")
                    for s in range(2)
                ]
                st["k"] = [
                    qkp.tile([128, 8, N], cdt, tag="qk", name=f"k_{next(_uid)}")
                    for s in range(2)
                ]
                st["v"] = None
                st["wv"] = []
                for kk in range(KT):
                    w = w8vp.tile([128, 8, 128], cdt, tag="wv", name=f"wv_{next(_uid)}")
                    nc.sync.dma_start(
                        out=w[:],
                        in_=wqkv[kk, 16:24].rearrange("n p f -> p n f"),
                    )
                    st["wv"].append(w)
                state[b] = st

            def ensure_v(b):
                st = state[b]
                if st["v"] is None:
                    st["v"] = []
                    for s in range(2):
                        v = vpp.tile(
                            [128, 5, H, D + 1], cdt, tag="v",
                            name=f"v_{next(_uid)}",
                        )
                        nc.vector.memset(v[:, :, :, D:D + 1], 1.0)
                        st["v"].append(v)

            def emit_qk_unit(b, n):
                st = state[b]
                wts = []
                for kk in range(KT):
                    w = w1p.tile([128, 128], cdt, tag="w1", name=f"wqk_{next(_uid)}")
                    nc.sync.dma_start(out=w[:], in_=wqkv[kk, n])
                    wts.append(w)
                for s in range(2):
                    dst = st["q"][s] if n < 8 else st["k"][s]
                    nd = n % 8
                    ps = psp.tile([128, 2, 512], F32, tag="sc", name=f"ps_{next(_uid)}")
                    for ci, (c0, cl) in enumerate(CH):
                        for kk in range(KT):
                            nc.tensor.matmul(
                                ps[:, ci, 0:cl],
                                lhsT=wts[kk],
                                rhs=st["xts"][s][:, kk, c0:c0 + cl],
                                start=(kk == 0),
                                stop=(kk == KT - 1),
                            )
                    nc.vector.tensor_copy(out=dst[:, nd, 0:289], in_=ps[:, 0, 0:289])
                    nc.vector.tensor_copy(out=dst[:, nd, 288:577], in_=ps[:, 1, 0:289])

            def emit_v_unit(b, s, ti):
                ensure_v(b)
                st = state[b]
                t0, tl = TOKT[ti]
                ps = psp.tile([128, 2, 512], F32, tag="sc", name=f"ps_{next(_uid)}")
                for ci in range(2):
                    for kk in range(KT):
                        nc.tensor.matmul(
                            ps[0:tl, ci, :],
                            lhsT=st["xts"][s][:, kk, t0:t0 + tl],
                            rhs=st["wv"][kk][:, 4 * ci:4 * ci + 4, :],
                            start=(kk == 0),
                            stop=(kk == KT - 1),
                        )
                for ci in range(2):
                    nc.vector.tensor_copy(
                        out=st["v"][s][0:tl, ti, 8 * ci:8 * ci + 8, 0:D],
                        in_=ps[0:tl, ci, :].rearrange("p (h d) -> p h d", d=D),
                    )

            def emit_attn_head(b, s, h, att):
                st = state[b]
                ensure_v(b)
                kv = s if h < HS else 1 - s
                par = (h % 2) * D
                nt = h // 2

                et = epp.tile(
                    [128, 5, 2, 289], cdt, tag="et", name=f"et_{next(_uid)}"
                )
                for jt, (j0, jl) in enumerate(TOKT):
                    ps = psp.tile(
                        [128, 2, 512], F32, tag="sc", name=f"ps_{next(_uid)}"
                    )
                    for ci, (c0, cl) in enumerate(CH):
                        nc.tensor.matmul(
                            ps[0:jl, ci, 0:cl],
                            lhsT=st["k"][kv][par:par + D, nt, j0:j0 + jl],
                            rhs=st["q"][s][par:par + D, nt, c0:c0 + cl],
                            start=True,
                            stop=True,
                        )
                    nc.scalar.activation(
                        out=et[0:jl, jt],
                        in_=ps[0:jl, :, 0:289],
                        func=mybir.ActivationFunctionType.Exp,
                        scale=SCALE,
                    )

                pa = psp.tile([128, 512], F32, tag="paA", name=f"pa_{next(_uid)}")
                pb = psp.tile([128, 512], F32, tag="paB", name=f"pb_{next(_uid)}")
                for jt, (j0, jl) in enumerate(TOKT):
                    nc.tensor.matmul(
                        pa[0:D + 1, 0:289],
                        lhsT=st["v"][kv][0:jl, jt, h, :],
                        rhs=et[0:jl, jt, 0],
                        start=(jt == 0),
                        stop=(jt == 4),
                    )
                    nc.tensor.matmul(
                        pb[0:D + 1, 0:289],
                        lhsT=st["v"][kv][0:jl, jt, h, :],
                        rhs=et[0:jl, jt, 1],
                        start=(jt == 0),
                        stop=(jt == 4),
                    )

                rb = rbpp.tile([128, N], F32, tag="rb", name=f"rb_{next(_uid)}")
                nc.vector.reciprocal(
                    out=rb[D:D + 1, 0:289], in_=pa[D:D + 1, 0:289]
                )
                nc.vector.reciprocal(
                    out=rb[D:D + 1, 288:577], in_=pb[D:D + 1, 0:289]
                )
                rd = drp.tile([N], F32, tag="rd", name=f"rd_{next(_uid)}")
                nc.sync.dma_start(out=rd[None, :], in_=rb[D:D + 1, :])
                nc.sync.dma_start(
                    out=rb[0:D, 0:289],
                    in_=rd[None, 0:289].to_broadcast([D, 289]),
                )
                nc.sync.dma_start(
                    out=rb[0:D, 289:577],
                    in_=rd[None, 289:577].to_broadcast([D, 288]),
                )
                if par == 0:
                    nc.vector.tensor_tensor(
                        out=att[0:D, nt, 0:289],
                        in0=pa[0:D, 0:289],
                        in1=rb[0:D, 0:289],
                        op=mybir.AluOpType.mult,
                    )
                    nc.vector.tensor_tensor(
                        out=att[0:D, nt, 288:577],
                        in0=pb[0:D, 0:289],
                        in1=rb[0:D, 288:577],
                        op=mybir.AluOpType.mult,
                    )
                else:
                    stg = stgp.tile([D, N], cdt, tag="stg", name=f"st_{next(_uid)}")
                    nc.vector.tensor_tensor(
                        out=stg[:, 0:289],
                        in0=pa[0:D, 0:289],
                        in1=rb[0:D, 0:289],
                        op=mybir.AluOpType.mult,
                    )
                    nc.vector.tensor_tensor(
                        out=stg[:, 288:577],
                        in0=pb[0:D, 0:289],
                        in1=rb[0:D, 288:577],
                        op=mybir.AluOpType.mult,
                    )
                    nc.sync.dma_start(out=att[D:128, nt, :], in_=stg[:])

            def emit_proj_unit(b, s, ti, att, wps):
                t0, tl = TOKT[ti]
                ps = psp.tile([128, 2, 512], F32, tag="sc", name=f"ps_{next(_uid)}")
                for ci in range(2):
                    for kk in range(KT):
                        nc.tensor.matmul(
                            ps[0:tl, ci, :],
                            lhsT=att[:, kk, t0:t0 + tl],
                            rhs=wps[kk][:, 4 * ci:4 * ci + 4, :],
                            start=(kk == 0),
                            stop=(kk == KT - 1),
                        )
                for ci in range(2):
                    ob = outp.tile([128, 512], F32, tag="ob", name=f"ob_{next(_uid)}")
                    nc.vector.tensor_tensor(
                        out=ob[0:tl, :],
                        in0=ps[0:tl, ci, :],
                        in1=bias_sb[0:tl, 512 * ci:512 * ci + 512],
                        op=mybir.AluOpType.add,
                    )
                    nc.sync.dma_start(
                        out=out[2 * b + s, t0:t0 + tl, 512 * ci:512 * ci + 512],
                        in_=ob[0:tl, :],
                    )

            loop_ctx = (
                tc.For_i(0, reps, 1) if reps > 1 else contextlib.nullcontext()
            )
            with loop_ctx:
                load_batch_inputs(0)
                for n in range(16):
                    emit_qk_unit(0, n)
                for s in range(2):
                    for ti in range(5):
                        emit_v_unit(0, s, ti)

                for b in range(BL):
                    st = state[b]
                    feeders = []
                    if b + 1 < BL:
                        load_batch_inputs(b + 1)
                        feeders += [("qk", n) for n in range(16)]
                    fi = 0

                    wps = []
                    if mode not in ("qkv", "noproj"):
                        for kk in range(KT):
                            w = w8pp.tile(
                                [128, 8, 128], cdt, tag="wp", name=f"wp_{next(_uid)}"
                            )
                            nc.sync.dma_start(
                                out=w[:], in_=wproj[kk].rearrange("n p f -> p n f")
                            )
                            wps.append(w)

                    att_sb = [None, None]
                    if mode != "qkv":
                        att_sb = []
                        for s in range(2):
                            att_sb.append(
                                xap.tile(
                                    [128, KT, N], cdt, tag="xa",
                                    name=f"att_{next(_uid)}",
                                )
                            )

                    proj_done = set()
                    for i, (s, h) in enumerate(
                        [(s, h) for s in range(2) for h in range(H)]
                    ):
                        if mode != "qkv":
                            emit_attn_head(b, s, h, att_sb[s])
                        if i % 2 == 0 and fi < len(feeders):
                            f = feeders[fi]
                            fi += 1
                            emit_qk_unit(b + 1, f[1])
                        elif (
                            mode == "full" and b + 1 >= BL and s == 1
                            and h >= H - 5
                        ):
                            ti = h - (H - 5)
                            emit_proj_unit(b, 0, ti, att_sb[0], wps)
                            proj_done.add((0, ti))
                    tail = (
                        [
                            ("p", s, ti)
                            for s in range(2)
                            for ti in range(5)
                            if (s, ti) not in proj_done
                        ]
                        if mode not in ("qkv", "noproj") else []
                    )
                    vunits = (
                        [("v", s, ti) for s in range(2) for ti in range(5)]
                        if b + 1 < BL else []
                    )
                    merged = []
                    for j in range(max(len(tail), len(vunits))):
                        if j < len(tail):
                            merged.append(tail[j])
                        if j < len(vunits):
                            merged.append(vunits[j])
                    for u in merged:
                        if u[0] == "p":
                            emit_proj_unit(b, u[1], u[2], att_sb[u[1]], wps)
                        else:
                            emit_v_unit(b + 1, u[1], u[2])
                    del state[b]

    split_excess_waits(nc)
    return nc


_CACHE = {}

CDT = mybir.dt.bfloat16  # compute dtype knob: bfloat16 | float32r | float32


def _get_nc(reps=1, mode="full"):
    key = (str(CDT), reps, mode)
    if key not in _CACHE:
        _CACHE[key] = build_kernel(CDT, reps=reps, mode=mode)
    return _CACHE[key]


def prep_in_maps(x1, x2, Wqkv, Wproj, bproj, cdt=None):
    cdt = cdt or CDT
    np_cdt = mybir.dt.np(cdt)
    x1 = np.asarray(x1, dtype=np.float32)
    x2 = np.asarray(x2, dtype=np.float32)
    Wqkv = np.asarray(Wqkv, dtype=np.float32)
    Wproj = np.asarray(Wproj, dtype=np.float32)
    bproj = np.asarray(bproj, dtype=np.float32)

    wq = np.ascontiguousarray(
        Wqkv.reshape(KT, 128, 24, 128).transpose(0, 2, 1, 3)
    ).astype(np_cdt)
    wp = np.ascontiguousarray(
        Wproj.reshape(KT, 128, 8, 128).transpose(0, 2, 1, 3)
    ).astype(np_cdt)
    biasr = np.ascontiguousarray(
        np.broadcast_to(bproj, (128, C))
    ).astype(np.float32)

    # [B, N, C] -> per-core [NSEQ, C, N], batch-major stream-minor
    xt_all = np.empty((NCORES, NSEQ, C, N), dtype=np_cdt)
    for c in range(NCORES):
        for lb in range(BL):
            b = BL * c + lb
            xt_all[c, 2 * lb + 0] = x1[b].T.astype(np_cdt)
            xt_all[c, 2 * lb + 1] = x2[b].T.astype(np_cdt)

    return [
        {"xt": xt_all[c], "wqkv": wq, "wproj": wp, "biasr": biasr}
        for c in range(NCORES)
    ]


def unpack_results(results):
    out1 = np.empty((NCORES * BL, N, C), dtype=np.float32)
    out2 = np.empty((NCORES * BL, N, C), dtype=np.float32)
    for c in range(NCORES):
        o = results[c]["out"]
        for lb in range(BL):
            out1[BL * c + lb] = o[2 * lb + 0]
            out2[BL * c + lb] = o[2 * lb + 1]
    return out1, out2


def kernel(x1, x2, Wqkv, Wproj, bproj):
    nc = _get_nc()
    in_maps = prep_in_maps(x1, x2, Wqkv, Wproj, bproj)
    res = run_bass_kernel_spmd(nc, in_maps, core_ids=list(range(NCORES)))
    return unpack_results(res.results)



# revision 26
# speedup vs baseline: 1.1932x; 1.1932x over previous
"""Trainium2 Bass kernel for dual-stream cross/self attention (nn_Attention).

Reference semantics (per batch b):
  qkv_s = x_s @ Wqkv  -> q_s,k_s,v_s  [H=16 heads, N=577 tokens, d=64]
  stream s output head h attends with q_s and (k_s,v_s) if h<10 else (k_o,v_o)
  out_s = concat_heads @ Wproj + bproj

Sharding: batch (16) data-parallel over 8 cores, 2 batches/core; weights
replicated. Per core, 4 sequences (2 batches x 2 streams) are processed.

Per-core dataflow (all matmuls contract over the SBUF partition dim):
  - host supplies x pre-transposed (xt: [c_in, tok]) and tile-packed weights
  - q,k computed as [c_out, tok] (c-major); v as [tok, c_out] (tok-major),
    with a constant ones-column appended per head so the attention matmul
    also produces the softmax denominator for free
  - scores^T[j,i] = k_head^T q_head, exp via ScalarE (scale fused, no max
    subtraction -- |score*scale| <= ~3.5 for this problem so exp is safe)
  - attn^T[d,i] (+ sumexp row) = [v_head|1]^T @ exp^T, accumulated over j
    one 289-token chunk at a time (chunk-outer) so each chunk uses a single
    PSUM bank
  - softmax denominators are broadcast across partitions with a 1-row PE
    outer-product matmul (lhsT=ones row at partition 64, rhs=recip row) --
    no DRAM round-trip DMAs; the normalize multiply reads both PSUM tiles
  - normalization of head h is emitted after the scores of head h+1 so the
    PE never stalls waiting for the DVE reciprocal
  - proj: out[tok,c] = attn^T-tiles^T @ Wproj-tiles, bias added from a
    host-replicated [128,1024] bias tile, one output DMA per token tile
"""

import numpy as np

import concourse.bass as bass
import concourse.mybir as mybir
import concourse.tile as tile
from concourse.bass_utils import run_bass_kernel_spmd

# ---------------------------------------------------------------------------
# Workaround: this walrus build rejects any instruction carrying >1 sem wait
# ("Too many sync wait commands").  Post-process the scheduled program and
# move excess waits onto single-wait NoOps inserted just before, on the same
# engine (engines execute their stream in order, so this is equivalent).
# ---------------------------------------------------------------------------


def split_excess_waits(nc, max_waits=1):
    cnt = 0
    for f in nc.m.functions:
        for blk in f.blocks:
            insts = blk.instructions
            need = any(
                inst.sync_info is not None
                and len(inst.sync_info.on_wait) > max_waits
                for inst in insts
            )
            if not need:
                continue
            newl = []
            for inst in insts:
                si = inst.sync_info
                if si is not None and len(si.on_wait) > max_waits:
                    waits = list(si.on_wait)
                    for w in waits[max_waits:]:
                        nop = mybir.InstNoOp(
                            name=f"wsplit_{cnt}",
                            engine=inst.engine,
                            ins=[],
                            outs=[],
                            sync_info=mybir.SyncInfo(on_wait=[w], on_update=[]),
                        )
                        cnt += 1
                        newl.append(nop)
                    si.on_wait = waits[:max_waits]
                newl.append(inst)
            blk.instructions = newl
    return cnt

# ---------------------------------------------------------------------------

F32 = mybir.dt.float32

N = 577          # tokens
C = 1024         # model dim
H = 16           # heads
D = 64           # head dim
HS = 10          # first HS heads self-attend, rest cross-attend
KT = 8           # c_in tiles of 128
SCALE = D ** -0.5
NCORES = 8
BL = 2           # local batches per core
NSEQ = 2 * BL    # sequences per core (batch-major, stream-minor)

# token partition tiles (start, len)
TOKT = [(0, 128), (128, 128), (256, 128), (384, 128), (512, 65)]
# token free-dim chunks (start, len): overlap 1 col at 288 so both are 289
# wide and a single ScalarE op can cover both PSUM sub-banks garbage-free
CH = [(0, 289), (288, 289)]

def build_kernel(cdt, reps=1, mode="full", split=True):
    nc = bass.Bass()
    # all inputs are packed host-side so every DMA lands per-partition
    # contiguous chunks >= 512B (smaller chunks halve DMA bus efficiency)
    xt = nc.dram_tensor("xt", [NSEQ, 128, KT, N], cdt, kind="ExternalInput")
    wqk = nc.dram_tensor("wqk", [16, 128, KT, 128], cdt, kind="ExternalInput")
    wvd = nc.dram_tensor("wvd", [128, KT, 8, 128], cdt, kind="ExternalInput")
    wproj = nc.dram_tensor("wproj", [128, KT, 8, 128], cdt, kind="ExternalInput")
    biasr = nc.dram_tensor("biasr", [128, C], F32, kind="ExternalInput")
    out = nc.dram_tensor("out", [NSEQ, N, C], F32, kind="ExternalOutput")

    import contextlib
    import itertools
    _uid = itertools.count()

    with tile.TileContext(nc) as tc:
        with (
            tc.tile_pool(name="const", bufs=1) as constp,
            tc.tile_pool(name="xa", bufs=4) as xap,       # xt + attnT share
            tc.tile_pool(name="qk", bufs=8) as qkp,       # q,k of 2 batches
            tc.tile_pool(name="vp", bufs=2) as vpp,
            tc.tile_pool(name="ep", bufs=3) as epp,
            tc.tile_pool(name="w1", bufs=4) as w1p,       # streamed qk weights
            tc.tile_pool(name="w8v", bufs=1) as w8vp,     # wv tile
            tc.tile_pool(name="w8p", bufs=1) as w8pp,     # wproj tile
            tc.tile_pool(name="rbp", bufs=2) as rbpp,     # recip rows
            tc.tile_pool(name="stg", bufs=2) as stgp,     # odd-head staging
            tc.tile_pool(name="op", bufs=2) as outp,
            tc.tile_pool(name="ps", bufs=2, space="PSUM") as psp,
            tc.tile_pool(name="pa", bufs=2, space="PSUM") as pap,
            tc.tile_pool(name="bc", bufs=2, space="PSUM") as bcp,
        ):
            bias_sb = constp.tile([128, C], F32, tag="bias")
            nc.sync.dma_start(out=bias_sb[:], in_=biasr[:])
            ones_sb = constp.tile([128, D], cdt, tag="ones")
            nc.vector.memset(ones_sb[D:D + 1, :], 1.0)

            state = {}

            def load_batch_inputs(b):
                st = {}
                st["xts"] = []
                for s in range(2):
                    t = xap.tile([128, KT, N], cdt, tag="xa", name=f"xt_{next(_uid)}")
                    nc.sync.dma_start(out=t[:], in_=xt[2 * b + s])
                    st["xts"].append(t)
                st["q"] = [
                    qkp.tile([128, 8, N], cdt, tag="qk", name=f"q_{next(_uid)}")
                    for s in range(2)
                ]
                st["k"] = [
                    qkp.tile([128, 8, N], cdt, tag="qk", name=f"k_{next(_uid)}")
                    for s in range(2)
                ]
                st["v"] = None
                state[b] = st

            def ensure_v(b):
                st = state[b]
                if st["v"] is None:
                    st["v"] = []
                    for s in range(2):
                        v = vpp.tile(
                            [128, 5, H, D + 1], cdt, tag="v",
                            name=f"v_{next(_uid)}",
                        )
                        nc.vector.memset(v[:, :, :, D:D + 1], 1.0)
                        st["v"].append(v)

            def emit_qk_unit(b, n):
                st = state[b]
                w = w1p.tile([128, KT, 128], cdt, tag="w1", name=f"wqk_{next(_uid)}")
                nc.sync.dma_start(out=w[:], in_=wqk[n])
                for s in range(2):
                    dst = st["q"][s] if n < 8 else st["k"][s]
                    nd = n % 8
                    ps = psp.tile([128, 2, 512], F32, tag="sc", name=f"ps_{next(_uid)}")
                    for ci, (c0, cl) in enumerate(CH):
                        for kk in range(KT):
                            nc.tensor.matmul(
                                ps[:, ci, 0:cl],
                                lhsT=w[:, kk, :],
                                rhs=st["xts"][s][:, kk, c0:c0 + cl],
                                start=(kk == 0),
                                stop=(kk == KT - 1),
                            )
                    # split the PSUM drain across two engines (gpsimd cannot
                    # read PSUM): these copies gate the sc ring feeding the PE
                    nc.vector.tensor_copy(out=dst[:, nd, 0:289], in_=ps[:, 0, 0:289])
                    nc.vector.tensor_copy(out=dst[:, nd, 288:577], in_=ps[:, 1, 0:289])

            def emit_v_unit(b, s, ti, wv):
                ensure_v(b)
                st = state[b]
                t0, tl = TOKT[ti]
                ps = psp.tile([128, 2, 512], F32, tag="sc", name=f"ps_{next(_uid)}")
                for ci in range(2):
                    for kk in range(KT):
                        nc.tensor.matmul(
                            ps[0:tl, ci, :],
                            lhsT=st["xts"][s][:, kk, t0:t0 + tl],
                            rhs=wv[:, kk, 4 * ci:4 * ci + 4, :],
                            start=(kk == 0),
                            stop=(kk == KT - 1),
                        )
                for ci in range(2):
                    nc.vector.tensor_copy(
                        out=st["v"][s][0:tl, ti, 8 * ci:8 * ci + 8, 0:D],
                        in_=ps[0:tl, ci, :].rearrange("p (h d) -> p h d", d=D),
                    )

            def score_tile(cinfo, jt):
                st = state[cinfo["b"]]
                kv, par, nt, s = (
                    cinfo["kv"], cinfo["par"], cinfo["nt"], cinfo["s"]
                )
                j0, jl = TOKT[jt]
                ps = psp.tile(
                    [128, 2, 512], F32, tag="sc", name=f"ps_{next(_uid)}"
                )
                for ci, (c0, cl) in enumerate(CH):
                    nc.tensor.matmul(
                        ps[0:jl, ci, 0:cl],
                        lhsT=st["k"][kv][par:par + D, nt, j0:j0 + jl],
                        rhs=st["q"][s][par:par + D, nt, c0:c0 + cl],
                        start=True,
                        stop=True,
                    )
                nc.scalar.activation(
                    out=cinfo["et"][0:jl, jt],
                    in_=ps[0:jl, :, 0:289],
                    func=mybir.ActivationFunctionType.Exp,
                    scale=SCALE,
                )

            def av_chunk(pinfo, ci):
                st = state[pinfo["b"]]
                pa = pap.tile([128, 512], F32, tag="pa", name=f"pa_{next(_uid)}")
                for jt, (j0, jl) in enumerate(TOKT):
                    nc.tensor.matmul(
                        pa[0:D + 1, 0:289],
                        lhsT=st["v"][pinfo["kv"]][0:jl, jt, pinfo["h"], :],
                        rhs=pinfo["et"][0:jl, jt, ci],
                        start=(jt == 0),
                        stop=(jt == 4),
                    )
                pinfo["pacc"].append(pa)

            def recip_chunk(pinfo, ci):
                if ci == 0:
                    pinfo["rb"] = rbpp.tile(
                        [128, N], cdt, tag="rb", name=f"rb_{next(_uid)}"
                    )
                c0 = (0, 288)[ci]
                with nc.allow_low_precision(
                    reason="softmax denom reciprocal to bf16: 0.2% "
                    "column-wise scale error, within tolerance"
                ):
                    nc.vector.reciprocal(
                        out=pinfo["rb"][D:D + 1, c0:c0 + 289],
                        in_=pinfo["pacc"][ci][D:D + 1, 0:289],
                    )

            def norm_head(pinfo):
                par, nt, att, rb = (
                    pinfo["par"], pinfo["nt"], pinfo["att"], pinfo["rb"]
                )
                paA, paB = pinfo["pacc"]
                bcA = bcp.tile([128, 512], F32, tag="bc", name=f"bc_{next(_uid)}")
                bcB = bcp.tile([128, 512], F32, tag="bc", name=f"bc_{next(_uid)}")
                nc.tensor.matmul(
                    bcA[0:D, 0:289],
                    lhsT=ones_sb[D:D + 1, :],
                    rhs=rb[D:D + 1, 0:289],
                    start=True,
                    stop=True,
                )
                nc.tensor.matmul(
                    bcB[0:D, 0:289],
                    lhsT=ones_sb[D:D + 1, :],
                    rhs=rb[D:D + 1, 288:577],
                    start=True,
                    stop=True,
                )
                # DVE cannot read two PSUM operands in one op: stage the
                # broadcast rows into the unused partitions of the rb tile
                # (SBUF), then multiply pa (PSUM) by rb (SBUF)
                nc.vector.tensor_copy(out=rb[0:D, 0:289], in_=bcA[0:D, 0:289])
                nc.vector.tensor_copy(out=rb[0:D, 288:577], in_=bcB[0:D, 0:289])
                if par == 0:
                    nc.vector.tensor_tensor(
                        out=att[0:D, nt, 0:289],
                        in0=paA[0:D, 0:289],
                        in1=rb[0:D, 0:289],
                        op=mybir.AluOpType.mult,
                    )
                    nc.vector.tensor_tensor(
                        out=att[0:D, nt, 288:577],
                        in0=paB[0:D, 0:289],
                        in1=rb[0:D, 288:577],
                        op=mybir.AluOpType.mult,
                    )
                else:
                    stg = stgp.tile([D, N], cdt, tag="stg", name=f"st_{next(_uid)}")
                    nc.vector.tensor_tensor(
                        out=stg[:, 0:289],
                        in0=paA[0:D, 0:289],
                        in1=rb[0:D, 0:289],
                        op=mybir.AluOpType.mult,
                    )
                    nc.vector.tensor_tensor(
                        out=stg[:, 288:577],
                        in0=paB[0:D, 0:289],
                        in1=rb[0:D, 288:577],
                        op=mybir.AluOpType.mult,
                    )
                    # Activation queue: by the time the ScalarE reaches
                    # this DMA its wait (the DVE mults above) has resolved,
                    # so it only costs the ~0.7us HWDGE issue slot
                    nc.scalar.dma_start(out=att[D:128, nt, :], in_=stg[:])

            def emit_head_pipeline(b, items, mids, att_sb):
                """Two-stage software pipeline over heads: iteration i emits
                the scores+exp of head i tile-interleaved with the AV +
                normalization of head i-1, so the ScalarE exp (657ns/tile)
                is never the gate on the 2-deep sc-PSUM ring (the PE only
                produces score tiles at less than half its burst rate)."""
                ensure_v(b)
                prev = None
                for i in range(len(items) + 1):
                    cinfo = None
                    if i < len(items):
                        s, h = items[i]
                        cinfo = {
                            "b": b, "s": s, "h": h,
                            "kv": s if h < HS else 1 - s,
                            "par": (h % 2) * D, "nt": h // 2,
                            "att": att_sb[s], "pacc": [],
                            "et": epp.tile(
                                [128, 5, 2, 289], cdt, tag="et",
                                name=f"et_{next(_uid)}",
                            ),
                        }
                        score_tile(cinfo, 0)
                        score_tile(cinfo, 1)
                    if prev is not None:
                        av_chunk(prev, 0)
                    if cinfo is not None:
                        score_tile(cinfo, 2)
                        score_tile(cinfo, 3)
                    if prev is not None:
                        recip_chunk(prev, 0)
                        av_chunk(prev, 1)
                    if cinfo is not None:
                        score_tile(cinfo, 4)
                    if prev is not None:
                        recip_chunk(prev, 1)
                        norm_head(prev)
                    mid = mids.get(i)
                    if mid is not None:
                        mid()
                    prev = cinfo

            def emit_proj_unit(b, s, ti, att, wp):
                t0, tl = TOKT[ti]
                ps = psp.tile([128, 2, 512], F32, tag="sc", name=f"ps_{next(_uid)}")
                for ci in range(2):
                    for kk in range(KT):
                        nc.tensor.matmul(
                            ps[0:tl, ci, :],
                            lhsT=att[:, kk, t0:t0 + tl],
                            rhs=wp[:, kk, 4 * ci:4 * ci + 4, :],
                            start=(kk == 0),
                            stop=(kk == KT - 1),
                        )
                ob = outp.tile([128, C], F32, tag="ob", name=f"ob_{next(_uid)}")
                for ci in range(2):
                    nc.vector.tensor_tensor(
                        out=ob[0:tl, 512 * ci:512 * ci + 512],
                        in0=ps[0:tl, ci, :],
                        in1=bias_sb[0:tl, 512 * ci:512 * ci + 512],
                        op=mybir.AluOpType.add,
                    )
                # SP queue is idle during the proj phases (no input loads
                # pending), so this dependent DMA cannot head-of-line block
                nc.sync.dma_start(
                    out=out[2 * b + s, t0:t0 + tl, :],
                    in_=ob[0:tl, :],
                )

            loop_ctx = (
                tc.For_i(0, reps, 1) if reps > 1 else contextlib.nullcontext()
            )
            with loop_ctx:
                load_batch_inputs(0)
                for n in range(4):
                    emit_qk_unit(0, n)
                wv = w8vp.tile([128, KT, 8, 128], cdt, tag="wv",
                               name=f"wv_{next(_uid)}")
                nc.sync.dma_start(out=wv[:], in_=wvd[:])
                for n in range(4, 16):
                    emit_qk_unit(0, n)
                wp = None
                if mode not in ("qkv", "noproj"):
                    wp = w8pp.tile([128, KT, 8, 128], cdt, tag="wp",
                                   name=f"wp_{next(_uid)}")
                    nc.sync.dma_start(out=wp[:], in_=wproj[:])
                for s in range(2):
                    for ti in range(5):
                        emit_v_unit(0, s, ti, wv)

                for b in range(BL):
                    st = state[b]
                    feeders = []
                    if b + 1 < BL:
                        load_batch_inputs(b + 1)
                        feeders += [("qk", n) for n in range(16)]
                    fi = 0

                    att_sb = [None, None]
                    if mode != "qkv":
                        att_sb = []
                        for s in range(2):
                            att_sb.append(
                                xap.tile(
                                    [128, KT, N], cdt, tag="xa",
                                    name=f"att_{next(_uid)}",
                                )
                            )

                    proj_done = set()
                    items = [(s, h) for s in range(2) for h in range(H)]
                    mids = {}
                    for i, (s, h) in enumerate(items):
                        if i % 2 == 0 and fi < len(feeders):
                            f = feeders[fi]
                            fi += 1
                            mids[i] = (lambda n=f[1]: emit_qk_unit(b + 1, n))
                        elif (
                            mode == "full" and b + 1 >= BL and s == 1
                            and h >= H - 5
                        ):
                            ti = h - (H - 5)
                            # AV of head i finishes in iteration i+1
                            mids[i + 1] = (lambda ti=ti: emit_proj_unit(
                                b, 0, ti, att_sb[0], wp))
                            proj_done.add((0, ti))
                    if mode != "qkv":
                        emit_head_pipeline(b, items, mids, att_sb)
                    else:
                        for i in sorted(mids):
                            mids[i]()
                    tail = (
                        [
                            ("p", s, ti)
                            for s in range(2)
                            for ti in range(5)
                            if (s, ti) not in proj_done
                        ]
                        if mode not in ("qkv", "noproj") else []
                    )
                    vunits = (
                        [("v", s, ti) for s in range(2) for ti in range(5)]
                        if b + 1 < BL else []
                    )
                    merged = []
                    for j in range(max(len(tail), len(vunits))):
                        if j < len(tail):
                            merged.append(tail[j])
                        if j < len(vunits):
                            merged.append(vunits[j])
                    for u in merged:
                        if u[0] == "p":
                            emit_proj_unit(b, u[1], u[2], att_sb[u[1]], wp)
                        else:
                            emit_v_unit(b + 1, u[1], u[2], wv)
                    del state[b]

    if split:
        split_excess_waits(nc)
    return nc


_CACHE = {}

CDT = mybir.dt.bfloat16  # compute dtype knob: bfloat16 | float32r | float32


def _get_nc(reps=1, mode="full"):
    key = (str(CDT), reps, mode)
    if key not in _CACHE:
        _CACHE[key] = build_kernel(CDT, reps=reps, mode=mode)
    return _CACHE[key]


def prep_in_maps(x1, x2, Wqkv, Wproj, bproj, cdt=None):
    cdt = cdt or CDT
    np_cdt = mybir.dt.np(cdt)
    x1 = np.asarray(x1, dtype=np.float32)
    x2 = np.asarray(x2, dtype=np.float32)
    Wqkv = np.asarray(Wqkv, dtype=np.float32)
    Wproj = np.asarray(Wproj, dtype=np.float32)
    bproj = np.asarray(bproj, dtype=np.float32)

    # wqk[n, p, kt, f] = Wqkv[kt*128+p, n*128+f]  (q,k output blocks)
    wqk = np.ascontiguousarray(
        Wqkv[:, : 2 * C].reshape(KT, 128, 16, 128).transpose(2, 1, 0, 3)
    ).astype(np_cdt)
    # wvd[p, kt, n, f] = Wqkv[kt*128+p, 2C + n*128+f]  (v output blocks)
    wvd = np.ascontiguousarray(
        Wqkv[:, 2 * C:].reshape(KT, 128, 8, 128).transpose(1, 0, 2, 3)
    ).astype(np_cdt)
    wp = np.ascontiguousarray(
        Wproj.reshape(KT, 128, 8, 128).transpose(1, 0, 2, 3)
    ).astype(np_cdt)
    biasr = np.ascontiguousarray(
        np.broadcast_to(bproj, (128, C))
    ).astype(np.float32)

    # [B, N, C] -> per-core [NSEQ, 128, KT, N]: xt[s, p, kt, n] = x[n, kt*128+p]
    xt_all = np.empty((NCORES, NSEQ, 128, KT, N), dtype=np_cdt)
    for c in range(NCORES):
        for lb in range(BL):
            b = BL * c + lb
            xt_all[c, 2 * lb + 0] = (
                x1[b].T.reshape(KT, 128, N).transpose(1, 0, 2).astype(np_cdt)
            )
            xt_all[c, 2 * lb + 1] = (
                x2[b].T.reshape(KT, 128, N).transpose(1, 0, 2).astype(np_cdt)
            )

    return [
        {"xt": xt_all[c], "wqk": wqk, "wvd": wvd, "wproj": wp, "biasr": biasr}
        for c in range(NCORES)
    ]


def unpack_results(results):
    out1 = np.empty((NCORES * BL, N, C), dtype=np.float32)
    out2 = np.empty((NCORES * BL, N, C), dtype=np.float32)
    for c in range(NCORES):
        o = results[c]["out"]
        for lb in range(BL):
            out1[BL * c + lb] = o[2 * lb + 0]
            out2[BL * c + lb] = o[2 * lb + 1]
    return out1, out2


def kernel(x1, x2, Wqkv, Wproj, bproj):
    nc = _get_nc()
    in_maps = prep_in_maps(x1, x2, Wqkv, Wproj, bproj)
    res = run_bass_kernel_spmd(nc, in_maps, core_ids=list(range(NCORES)))
    return unpack_results(res.results)


# revision 30
# speedup vs baseline: 1.2486x; 1.0464x over previous
"""Trainium2 Bass kernel for dual-stream cross/self attention (nn_Attention).

Reference semantics (per batch b):
  qkv_s = x_s @ Wqkv  -> q_s,k_s,v_s  [H=16 heads, N=577 tokens, d=64]
  stream s output head h attends with q_s and (k_s,v_s) if h<10 else (k_o,v_o)
  out_s = concat_heads @ Wproj + bproj

Sharding: batch (16) data-parallel over 8 cores, 2 batches/core; weights
replicated. Per core, 4 sequences (2 batches x 2 streams) are processed.

Per-core dataflow (all matmuls contract over the SBUF partition dim):
  - host supplies x pre-transposed (xt: [c_in, tok]) and tile-packed weights
  - q,k computed as [c_out, tok] (c-major); v as [tok, c_out] (tok-major),
    with a constant ones-column appended per head so the attention matmul
    also produces the softmax denominator for free
  - scores^T[j,i] = k_head^T q_head, exp via ScalarE (scale fused, no max
    subtraction -- |score*scale| <= ~3.5 for this problem so exp is safe)
  - attn^T[d,i] (+ sumexp row) = [v_head|1]^T @ exp^T, accumulated over j
  - normalize along tokens via reciprocal + DMA partition-broadcast + mult
  - proj: out[tok,c] = attn^T-tiles^T @ Wproj-tiles, bias added from a
    host-replicated [128,1024] bias tile
"""

import numpy as np

import concourse.bass as bass
import concourse.mybir as mybir
import concourse.tile as tile
from concourse.bass_utils import run_bass_kernel_spmd

# ---------------------------------------------------------------------------
# Workaround: this walrus build rejects any instruction carrying >1 sem wait
# ("Too many sync wait commands").  Post-process the scheduled program and
# move excess waits onto single-wait NoOps inserted just before, on the same
# engine (engines execute their stream in order, so this is equivalent).
# ---------------------------------------------------------------------------


def split_excess_waits(nc, max_waits=1):
    cnt = 0
    for f in nc.m.functions:
        for blk in f.blocks:
            insts = blk.instructions
            need = any(
                inst.sync_info is not None
                and len(inst.sync_info.on_wait) > max_waits
                for inst in insts
            )
            if not need:
                continue
            newl = []
            for inst in insts:
                si = inst.sync_info
                if si is not None and len(si.on_wait) > max_waits:
                    waits = list(si.on_wait)
                    for w in waits[max_waits:]:
                        nop = mybir.InstNoOp(
                            name=f"wsplit_{cnt}",
                            engine=inst.engine,
                            ins=[],
                            outs=[],
                            sync_info=mybir.SyncInfo(on_wait=[w], on_update=[]),
                        )
                        cnt += 1
                        newl.append(nop)
                    si.on_wait = waits[:max_waits]
                newl.append(inst)
            blk.instructions = newl
    return cnt

# ---------------------------------------------------------------------------

F32 = mybir.dt.float32

N = 577          # tokens
C = 1024         # model dim
H = 16           # heads
D = 64           # head dim
HS = 10          # first HS heads self-attend, rest cross-attend
KT = 8           # c_in tiles of 128
SCALE = D ** -0.5
NCORES = 8
BL = 2           # local batches per core
NSEQ = 2 * BL    # sequences per core (batch-major, stream-minor)

# token partition tiles (start, len)
TOKT = [(0, 128), (128, 128), (256, 128), (384, 128), (512, 65)]
# token free-dim chunks (start, len): overlap 1 col at 288 so both are 289
# wide and a single ScalarE op can cover both PSUM sub-banks garbage-free
CH = [(0, 289), (288, 289)]

def build_kernel(cdt, reps=1, mode="full"):
    nc = bass.Bass()
    xt = nc.dram_tensor("xt", [NSEQ, C, N], cdt, kind="ExternalInput")
    wqkv = nc.dram_tensor("wqkv", [KT, 24, 128, 128], cdt, kind="ExternalInput")
    wproj = nc.dram_tensor("wproj", [KT, 8, 128, 128], cdt, kind="ExternalInput")
    biasr = nc.dram_tensor("biasr", [128, C], F32, kind="ExternalInput")
    out = nc.dram_tensor("out", [NSEQ, N, C], F32, kind="ExternalOutput")

    import contextlib
    import itertools
    _uid = itertools.count()

    with tile.TileContext(nc) as tc:
        with (
            tc.tile_pool(name="const", bufs=1) as constp,
            tc.tile_pool(name="xa", bufs=4) as xap,       # xt + attnT share
            tc.tile_pool(name="qk", bufs=8) as qkp,       # q,k of 2 batches
            tc.tile_pool(name="vp", bufs=2) as vpp,
            tc.tile_pool(name="ep", bufs=4) as epp,
            tc.tile_pool(name="w1", bufs=16) as w1p,      # streamed qk weights
            tc.tile_pool(name="w8v", bufs=8) as w8vp,     # wv tiles
            tc.tile_pool(name="w8p", bufs=8) as w8pp,     # wproj tiles
            tc.tile_pool(name="rbp", bufs=4) as rbpp,     # recip + broadcast
            tc.tile_pool(name="stg", bufs=2) as stgp,     # odd-head staging
            tc.tile_pool(name="op", bufs=2) as outp,
            tc.tile_pool(name="dr", bufs=4, space="DRAM") as drp,
            tc.tile_pool(name="ps", bufs=2, space="PSUM") as psp,
        ):
            bias_sb = constp.tile([128, C], F32, tag="bias")
            nc.sync.dma_start(out=bias_sb[:], in_=biasr[:])

            state = {}

            def load_batch_inputs(b):
                st = {}
                st["xts"] = []
                for s in range(2):
                    t = xap.tile([128, KT, N], cdt, tag="xa", name=f"xt_{next(_uid)}")
                    nc.sync.dma_start(
                        out=t[:],
                        in_=xt[2 * b + s].rearrange("(kt p) n -> p kt n", p=128),
                    )
                    st["xts"].append(t)
                st["q"] = [
                    qkp.tile([128, 8, N], cdt, tag="qk", name=f"q_{next(_uid)}")
                    for s in range(2)
                ]
                st["k"] = [
                    qkp.tile([128, 8, N], cdt, tag="qk", name=f"k_{next(_uid)}")
                    for s in range(2)
                ]
                st["v"] = None
                st["wv"] = []
                for kk in range(KT):
                    w = w8vp.tile([128, 8, 128], cdt, tag="wv", name=f"wv_{next(_uid)}")
                    nc.sync.dma_start(
                        out=w[:],
                        in_=wqkv[kk, 16:24].rearrange("n p f -> p n f"),
                    )
                    st["wv"].append(w)
                state[b] = st

            def ensure_v(b):
                st = state[b]
                if st["v"] is None:
                    st["v"] = []
                    for s in range(2):
                        v = vpp.tile(
                            [128, 5, H, D + 1], cdt, tag="v",
                            name=f"v_{next(_uid)}",
                        )
                        nc.vector.memset(v[:, :, :, D:D + 1], 1.0)
                        st["v"].append(v)

            def emit_qk_unit(b, n):
                st = state[b]
                wts = []
                for kk in range(KT):
                    w = w1p.tile([128, 128], cdt, tag="w1", name=f"wqk_{next(_uid)}")
                    nc.sync.dma_start(out=w[:], in_=wqkv[kk, n])
                    wts.append(w)
                for s in range(2):
                    dst = st["q"][s] if n < 8 else st["k"][s]
                    nd = n % 8
                    ps = psp.tile([128, 2, 512], F32, tag="sc", name=f"ps_{next(_uid)}")
                    for ci, (c0, cl) in enumerate(CH):
                        for kk in range(KT):
                            nc.tensor.matmul(
                                ps[:, ci, 0:cl],
                                lhsT=wts[kk],
                                rhs=st["xts"][s][:, kk, c0:c0 + cl],
                                start=(kk == 0),
                                stop=(kk == KT - 1),
                            )
                    nc.vector.tensor_copy(out=dst[:, nd, 0:289], in_=ps[:, 0, 0:289])
                    nc.vector.tensor_copy(out=dst[:, nd, 288:577], in_=ps[:, 1, 0:289])

            def emit_v_unit(b, s, ti):
                ensure_v(b)
                st = state[b]
                t0, tl = TOKT[ti]
                ps = psp.tile([128, 2, 512], F32, tag="sc", name=f"ps_{next(_uid)}")
                for ci in range(2):
                    for kk in range(KT):
                        nc.tensor.matmul(
                            ps[0:tl, ci, :],
                            lhsT=st["xts"][s][:, kk, t0:t0 + tl],
                            rhs=st["wv"][kk][:, 4 * ci:4 * ci + 4, :],
                            start=(kk == 0),
                            stop=(kk == KT - 1),
                        )
                for ci in range(2):
                    nc.vector.tensor_copy(
                        out=st["v"][s][0:tl, ti, 8 * ci:8 * ci + 8, 0:D],
                        in_=ps[0:tl, ci, :].rearrange("p (h d) -> p h d", d=D),
                    )

            def emit_attn_head(b, s, h, att):
                st = state[b]
                ensure_v(b)
                kv = s if h < HS else 1 - s
                par = (h % 2) * D
                nt = h // 2

                et = epp.tile(
                    [128, 5, 2, 289], cdt, tag="et", name=f"et_{next(_uid)}"
                )
                for jt, (j0, jl) in enumerate(TOKT):
                    ps = psp.tile(
                        [128, 2, 512], F32, tag="sc", name=f"ps_{next(_uid)}"
                    )
                    for ci, (c0, cl) in enumerate(CH):
                        nc.tensor.matmul(
                            ps[0:jl, ci, 0:cl],
                            lhsT=st["k"][kv][par:par + D, nt, j0:j0 + jl],
                            rhs=st["q"][s][par:par + D, nt, c0:c0 + cl],
                            start=True,
                            stop=True,
                        )
                    nc.scalar.activation(
                        out=et[0:jl, jt],
                        in_=ps[0:jl, :, 0:289],
                        func=mybir.ActivationFunctionType.Exp,
                        scale=SCALE,
                    )

                pa = psp.tile([128, 512], F32, tag="paA", name=f"pa_{next(_uid)}")
                pb = psp.tile([128, 512], F32, tag="paB", name=f"pb_{next(_uid)}")
                for jt, (j0, jl) in enumerate(TOKT):
                    nc.tensor.matmul(
                        pa[0:D + 1, 0:289],
                        lhsT=st["v"][kv][0:jl, jt, h, :],
                        rhs=et[0:jl, jt, 0],
                        start=(jt == 0),
                        stop=(jt == 4),
                    )
                    nc.tensor.matmul(
                        pb[0:D + 1, 0:289],
                        lhsT=st["v"][kv][0:jl, jt, h, :],
                        rhs=et[0:jl, jt, 1],
                        start=(jt == 0),
                        stop=(jt == 4),
                    )

                rb = rbpp.tile([128, N], F32, tag="rb", name=f"rb_{next(_uid)}")
                nc.vector.reciprocal(
                    out=rb[D:D + 1, 0:289], in_=pa[D:D + 1, 0:289]
                )
                nc.vector.reciprocal(
                    out=rb[D:D + 1, 288:577], in_=pb[D:D + 1, 0:289]
                )
                rd = drp.tile([N], F32, tag="rd", name=f"rd_{next(_uid)}")
                nc.sync.dma_start(out=rd[None, :], in_=rb[D:D + 1, :])
                nc.sync.dma_start(
                    out=rb[0:D, 0:289],
                    in_=rd[None, 0:289].to_broadcast([D, 289]),
                )
                nc.sync.dma_start(
                    out=rb[0:D, 289:577],
                    in_=rd[None, 289:577].to_broadcast([D, 288]),
                )
                if par == 0:
                    nc.vector.tensor_tensor(
                        out=att[0:D, nt, 0:289],
                        in0=pa[0:D, 0:289],
                        in1=rb[0:D, 0:289],
                        op=mybir.AluOpType.mult,
                    )
                    nc.vector.tensor_tensor(
                        out=att[0:D, nt, 288:577],
                        in0=pb[0:D, 0:289],
                        in1=rb[0:D, 288:577],
                        op=mybir.AluOpType.mult,
                    )
                else:
                    stg = stgp.tile([D, N], cdt, tag="stg", name=f"st_{next(_uid)}")
                    nc.vector.tensor_tensor(
                        out=stg[:, 0:289],
                        in0=pa[0:D, 0:289],
                        in1=rb[0:D, 0:289],
                        op=mybir.AluOpType.mult,
                    )
                    nc.vector.tensor_tensor(
                        out=stg[:, 288:577],
                        in0=pb[0:D, 0:289],
                        in1=rb[0:D, 288:577],
                        op=mybir.AluOpType.mult,
                    )
                    nc.sync.dma_start(out=att[D:128, nt, :], in_=stg[:])

            def emit_proj_unit(b, s, ti, att, wps):
                t0, tl = TOKT[ti]
                ps = psp.tile([128, 2, 512], F32, tag="sc", name=f"ps_{next(_uid)}")
                for ci in range(2):
                    for kk in range(KT):
                        nc.tensor.matmul(
                            ps[0:tl, ci, :],
                            lhsT=att[:, kk, t0:t0 + tl],
                            rhs=wps[kk][:, 4 * ci:4 * ci + 4, :],
                            start=(kk == 0),
                            stop=(kk == KT - 1),
                        )
                for ci in range(2):
                    ob = outp.tile([128, 512], F32, tag="ob", name=f"ob_{next(_uid)}")
                    nc.vector.tensor_tensor(
                        out=ob[0:tl, :],
                        in0=ps[0:tl, ci, :],
                        in1=bias_sb[0:tl, 512 * ci:512 * ci + 512],
                        op=mybir.AluOpType.add,
                    )
                    nc.sync.dma_start(
                        out=out[2 * b + s, t0:t0 + tl, 512 * ci:512 * ci + 512],
                        in_=ob[0:tl, :],
                    )

            loop_ctx = (
                tc.For_i(0, reps, 1) if reps > 1 else contextlib.nullcontext()
            )
            with loop_ctx:
                load_batch_inputs(0)
                for n in range(16):
                    emit_qk_unit(0, n)
                for s in range(2):
                    for ti in range(5):
                        emit_v_unit(0, s, ti)

                for b in range(BL):
                    st = state[b]
                    feeders = []
                    if b + 1 < BL:
                        load_batch_inputs(b + 1)
                        feeders += [("qk", n) for n in range(16)]
                    fi = 0

                    wps = []
                    if mode not in ("qkv", "noproj"):
                        for kk in range(KT):
                            w = w8pp.tile(
                                [128, 8, 128], cdt, tag="wp", name=f"wp_{next(_uid)}"
                            )
                            nc.sync.dma_start(
                                out=w[:], in_=wproj[kk].rearrange("n p f -> p n f")
                            )
                            wps.append(w)

                    att_sb = [None, None]
                    if mode != "qkv":
                        att_sb = []
                        for s in range(2):
                            att_sb.append(
                                xap.tile(
                                    [128, KT, N], cdt, tag="xa",
                                    name=f"att_{next(_uid)}",
                                )
                            )

                    proj_done = set()
                    for i, (s, h) in enumerate(
                        [(s, h) for s in range(2) for h in range(H)]
                    ):
                        if mode != "qkv":
                            emit_attn_head(b, s, h, att_sb[s])
                        if i % 2 == 0 and fi < len(feeders):
                            f = feeders[fi]
                            fi += 1
                            emit_qk_unit(b + 1, f[1])
                        elif (
                            mode == "full" and b + 1 >= BL and s == 1
                            and h >= H - 5
                        ):
                            ti = h - (H - 5)
                            emit_proj_unit(b, 0, ti, att_sb[0], wps)
                            proj_done.add((0, ti))
                    tail = (
                        [
                            ("p", s, ti)
                            for s in range(2)
                            for ti in range(5)
                            if (s, ti) not in proj_done
                        ]
                        if mode not in ("qkv", "noproj") else []
                    )
                    vunits = (
                        [("v", s, ti) for s in range(2) for ti in range(5)]
                        if b + 1 < BL else []
                    )
                    merged = []
                    for j in range(max(len(tail), len(vunits))):
                        if j < len(tail):
                            merged.append(tail[j])
                        if j < len(vunits):
                            merged.append(vunits[j])
                    for u in merged:
                        if u[0] == "p":
                            emit_proj_unit(b, u[1], u[2], att_sb[u[1]], wps)
                        else:
                            emit_v_unit(b + 1, u[1], u[2])
                    del state[b]

    split_excess_waits(nc)
    return nc


_CACHE = {}

CDT = mybir.dt.bfloat16  # compute dtype knob: bfloat16 | float32r | float32


def _get_nc(reps=1, mode="full"):
    key = (str(CDT), reps, mode)
    if key not in _CACHE:
        _CACHE[key] = build_kernel(CDT, reps=reps, mode=mode)
    return _CACHE[key]


def prep_in_maps(x1, x2, Wqkv, Wproj, bproj, cdt=None):
    cdt = cdt or CDT
    np_cdt = mybir.dt.np(cdt)
    x1 = np.asarray(x1, dtype=np.float32)
    x2 = np.asarray(x2, dtype=np.float32)
    Wqkv = np.asarray(Wqkv, dtype=np.float32)
    Wproj = np.asarray(Wproj, dtype=np.float32)
    bproj = np.asarray(bproj, dtype=np.float32)

    wq = np.ascontiguousarray(
        Wqkv.reshape(KT, 128, 24, 128).transpose(0, 2, 1, 3)
    ).astype(np_cdt)
    wp = np.ascontiguousarray(
        Wproj.reshape(KT, 128, 8, 128).transpose(0, 2, 1, 3)
    ).astype(np_cdt)
    biasr = np.ascontiguousarray(
        np.broadcast_to(bproj, (128, C))
    ).astype(np.float32)

    # [B, N, C] -> per-core [NSEQ, C, N], batch-major stream-minor
    xt_all = np.empty((NCORES, NSEQ, C, N), dtype=np_cdt)
    for c in range(NCORES):
        for lb in range(BL):
            b = BL * c + lb
            xt_all[c, 2 * lb + 0] = x1[b].T.astype(np_cdt)
            xt_all[c, 2 * lb + 1] = x2[b].T.astype(np_cdt)

    return [
        {"xt": xt_all[c], "wqkv": wq, "wproj": wp, "biasr": biasr}
        for c in range(NCORES)
    ]


def unpack_results(results):
    out1 = np.empty((NCORES * BL, N, C), dtype=np.float32)
    out2 = np.empty((NCORES * BL, N, C), dtype=np.float32)
    for c in range(NCORES):
        o = results[c]["out"]
        for lb in range(BL):
            out1[BL * c + lb] = o[2 * lb + 0]
            out2[BL * c + lb] = o[2 * lb + 1]
    return out1, out2


def kernel(x1, x2, Wqkv, Wproj, bproj):
    nc = _get_nc()
    in_maps = prep_in_maps(x1, x2, Wqkv, Wproj, bproj)
    res = run_bass_kernel_spmd(nc, in_maps, core_ids=list(range(NCORES)))
    return unpack_results(res.results)



# revision 32
# speedup vs baseline: 1.5916x; 1.2747x over previous
"""Trainium2 Bass kernel for dual-stream cross/self attention (nn_Attention).

Reference semantics (per batch b):
  qkv_s = x_s @ Wqkv  -> q_s,k_s,v_s  [H=16 heads, N=577 tokens, d=64]
  stream s output head h attends with q_s and (k_s,v_s) if h<10 else (k_o,v_o)
  out_s = concat_heads @ Wproj + bproj

Sharding: batch (16) data-parallel over 8 cores, 2 batches/core; weights
replicated. Per core, 4 sequences (2 batches x 2 streams) are processed.

This is the baseline dataflow/emission order (which measures fastest on HW)
with only the DMA layer reworked:
  - host packs every input so each DMA lands per-partition contiguous
    chunks >= 512B (sub-512B chunks halve DMA bus efficiency)
  - one DMA per qk weight unit ([128, KT, 128]) instead of 8 tile loads
  - wv / wproj loaded once per rep (single DMA each) instead of per batch
  - one output DMA per proj token tile ([tl, 1024]) instead of two
This cuts the per-rep DMA count from ~557 to ~290 without touching the PE
instruction mix, the softmax normalization, or the head schedule.

Per-core dataflow (all matmuls contract over the SBUF partition dim):
  - q,k computed as [c_out, tok] (c-major); v as [tok, c_out] (tok-major),
    with a constant ones-column appended per head so the attention matmul
    also produces the softmax denominator for free
  - scores^T[j,i] = k_head^T q_head, exp via ScalarE (scale fused, no max
    subtraction -- |score*scale| <= ~3.5 for this problem so exp is safe)
  - attn^T[d,i] (+ sumexp row) = [v_head|1]^T @ exp^T, accumulated over j
  - normalize along tokens via reciprocal + DMA partition-broadcast + mult
  - proj: out[tok,c] = attn^T-tiles^T @ Wproj-tiles, bias added from a
    host-replicated [128,1024] bias tile
"""

import numpy as np

import concourse.bass as bass
import concourse.mybir as mybir
import concourse.tile as tile
from concourse.bass_utils import run_bass_kernel_spmd

# ---------------------------------------------------------------------------
# Workaround: this walrus build rejects any instruction carrying >1 sem wait
# ("Too many sync wait commands").  Post-process the scheduled program and
# move excess waits onto single-wait NoOps inserted just before, on the same
# engine (engines execute their stream in order, so this is equivalent).
# ---------------------------------------------------------------------------


def split_excess_waits(nc, max_waits=1):
    cnt = 0
    for f in nc.m.functions:
        for blk in f.blocks:
            insts = blk.instructions
            need = any(
                inst.sync_info is not None
                and len(inst.sync_info.on_wait) > max_waits
                for inst in insts
            )
            if not need:
                continue
            newl = []
            for inst in insts:
                si = inst.sync_info
                if si is not None and len(si.on_wait) > max_waits:
                    waits = list(si.on_wait)
                    for w in waits[max_waits:]:
                        nop = mybir.InstNoOp(
                            name=f"wsplit_{cnt}",
                            engine=inst.engine,
                            ins=[],
                            outs=[],
                            sync_info=mybir.SyncInfo(on_wait=[w], on_update=[]),
                        )
                        cnt += 1
                        newl.append(nop)
                    si.on_wait = waits[:max_waits]
                newl.append(inst)
            blk.instructions = newl
    return cnt

# ---------------------------------------------------------------------------

F32 = mybir.dt.float32

N = 577          # tokens
C = 1024         # model dim
H = 16           # heads
D = 64           # head dim
HS = 10          # first HS heads self-attend, rest cross-attend
KT = 8           # c_in tiles of 128
SCALE = D ** -0.5
NCORES = 8
BL = 2           # local batches per core
NSEQ = 2 * BL    # sequences per core (batch-major, stream-minor)

# token partition tiles (start, len)
TOKT = [(0, 128), (128, 128), (256, 128), (384, 128), (512, 65)]
# token free-dim chunks (start, len): overlap 1 col at 288 so both are 289
# wide and a single ScalarE op can cover both PSUM sub-banks garbage-free
CH = [(0, 289), (288, 289)]

def build_kernel(cdt, reps=1, mode="full", split=True):
    nc = bass.Bass()
    xt = nc.dram_tensor("xt", [NSEQ, 128, KT, N], cdt, kind="ExternalInput")
    wqk = nc.dram_tensor("wqk", [16, 128, KT, 128], cdt, kind="ExternalInput")
    wvd = nc.dram_tensor("wvd", [128, KT, 8, 128], cdt, kind="ExternalInput")
    wproj = nc.dram_tensor("wproj", [128, KT, 8, 128], cdt, kind="ExternalInput")
    biasr = nc.dram_tensor("biasr", [128, C], F32, kind="ExternalInput")
    out = nc.dram_tensor("out", [NSEQ, N, C], F32, kind="ExternalOutput")

    import contextlib
    import itertools
    _uid = itertools.count()

    with tile.TileContext(nc) as tc:
        with (
            tc.tile_pool(name="const", bufs=1) as constp,
            tc.tile_pool(name="xa", bufs=4) as xap,       # xt + attnT share
            tc.tile_pool(name="qk", bufs=8) as qkp,       # q,k of 2 batches
            tc.tile_pool(name="vp", bufs=2) as vpp,
            tc.tile_pool(name="ep", bufs=3) as epp,
            tc.tile_pool(name="w1", bufs=3) as w1p,       # streamed qk weights
            tc.tile_pool(name="w8v", bufs=1) as w8vp,     # wv tile
            tc.tile_pool(name="w8p", bufs=1) as w8pp,     # wproj tile
            tc.tile_pool(name="rbp", bufs=4) as rbpp,     # recip + broadcast
            tc.tile_pool(name="stg", bufs=2) as stgp,     # odd-head staging
            tc.tile_pool(name="op", bufs=2) as outp,
            tc.tile_pool(name="dr", bufs=4, space="DRAM") as drp,
            tc.tile_pool(name="ps", bufs=2, space="PSUM") as psp,
        ):
            bias_sb = constp.tile([128, C], F32, tag="bias")
            nc.sync.dma_start(out=bias_sb[:], in_=biasr[:])

            state = {}

            def load_batch_inputs(b):
                st = {}
                st["xts"] = []
                for s in range(2):
                    t = xap.tile([128, KT, N], cdt, tag="xa", name=f"xt_{next(_uid)}")
                    nc.sync.dma_start(out=t[:], in_=xt[2 * b + s])
                    st["xts"].append(t)
                st["q"] = [
                    qkp.tile([128, 8, N], cdt, tag="qk", name=f"q_{next(_uid)}")
                    for s in range(2)
                ]
                st["k"] = [
                    qkp.tile([128, 8, N], cdt, tag="qk", name=f"k_{next(_uid)}")
                    for s in range(2)
                ]
                st["v"] = None
                state[b] = st

            def ensure_v(b):
                st = state[b]
                if st["v"] is None:
                    st["v"] = []
                    for s in range(2):
                        v = vpp.tile(
                            [128, 5, H, D + 1], cdt, tag="v",
                            name=f"v_{next(_uid)}",
                        )
                        nc.vector.memset(v[:, :, :, D:D + 1], 1.0)
                        st["v"].append(v)

            def emit_qk_unit(b, n):
                st = state[b]
                w = w1p.tile([128, KT, 128], cdt, tag="w1", name=f"wqk_{next(_uid)}")
                nc.sync.dma_start(out=w[:], in_=wqk[n])
                for s in range(2):
                    dst = st["q"][s] if n < 8 else st["k"][s]
                    nd = n % 8
                    ps = psp.tile([128, 2, 512], F32, tag="sc", name=f"ps_{next(_uid)}")
                    for ci, (c0, cl) in enumerate(CH):
                        for kk in range(KT):
                            nc.tensor.matmul(
                                ps[:, ci, 0:cl],
                                lhsT=w[:, kk, :],
                                rhs=st["xts"][s][:, kk, c0:c0 + cl],
                                start=(kk == 0),
                                stop=(kk == KT - 1),
                            )
                    nc.vector.tensor_copy(out=dst[:, nd, 0:289], in_=ps[:, 0, 0:289])
                    nc.vector.tensor_copy(out=dst[:, nd, 288:577], in_=ps[:, 1, 0:289])

            def emit_v_unit(b, s, ti, wv):
                ensure_v(b)
                st = state[b]
                t0, tl = TOKT[ti]
                ps = psp.tile([128, 2, 512], F32, tag="sc", name=f"ps_{next(_uid)}")
                for ci in range(2):
                    for kk in range(KT):
                        nc.tensor.matmul(
                            ps[0:tl, ci, :],
                            lhsT=st["xts"][s][:, kk, t0:t0 + tl],
                            rhs=wv[:, kk, 4 * ci:4 * ci + 4, :],
                            start=(kk == 0),
                            stop=(kk == KT - 1),
                        )
                for ci in range(2):
                    nc.vector.tensor_copy(
                        out=st["v"][s][0:tl, ti, 8 * ci:8 * ci + 8, 0:D],
                        in_=ps[0:tl, ci, :].rearrange("p (h d) -> p h d", d=D),
                    )

            def emit_attn_head(b, s, h, att):
                st = state[b]
                ensure_v(b)
                kv = s if h < HS else 1 - s
                par = (h % 2) * D
                nt = h // 2

                et = epp.tile(
                    [128, 5, 2, 289], cdt, tag="et", name=f"et_{next(_uid)}"
                )
                for jt, (j0, jl) in enumerate(TOKT):
                    ps = psp.tile(
                        [128, 2, 512], F32, tag="sc", name=f"ps_{next(_uid)}"
                    )
                    for ci, (c0, cl) in enumerate(CH):
                        nc.tensor.matmul(
                            ps[0:jl, ci, 0:cl],
                            lhsT=st["k"][kv][par:par + D, nt, j0:j0 + jl],
                            rhs=st["q"][s][par:par + D, nt, c0:c0 + cl],
                            start=True,
                            stop=True,
                        )
                    nc.scalar.activation(
                        out=et[0:jl, jt],
                        in_=ps[0:jl, :, 0:289],
                        func=mybir.ActivationFunctionType.Exp,
                        scale=SCALE,
                    )

                pa = psp.tile([128, 512], F32, tag="paA", name=f"pa_{next(_uid)}")
                pb = psp.tile([128, 512], F32, tag="paB", name=f"pb_{next(_uid)}")
                for jt, (j0, jl) in enumerate(TOKT):
                    nc.tensor.matmul(
                        pa[0:D + 1, 0:289],
                        lhsT=st["v"][kv][0:jl, jt, h, :],
                        rhs=et[0:jl, jt, 0],
                        start=(jt == 0),
                        stop=(jt == 4),
                    )
                    nc.tensor.matmul(
                        pb[0:D + 1, 0:289],
                        lhsT=st["v"][kv][0:jl, jt, h, :],
                        rhs=et[0:jl, jt, 1],
                        start=(jt == 0),
                        stop=(jt == 4),
                    )

                rb = rbpp.tile([128, N], F32, tag="rb", name=f"rb_{next(_uid)}")
                nc.vector.reciprocal(
                    out=rb[D:D + 1, 0:289], in_=pa[D:D + 1, 0:289]
                )
                nc.vector.reciprocal(
                    out=rb[D:D + 1, 288:577], in_=pb[D:D + 1, 0:289]
                )
                rd = drp.tile([N], F32, tag="rd", name=f"rd_{next(_uid)}")
                nc.sync.dma_start(out=rd[None, :], in_=rb[D:D + 1, :])
                nc.sync.dma_start(
                    out=rb[0:D, 0:577],
                    in_=rd[None, 0:577].to_broadcast([D, 577]),
                )
                if par == 0:
                    nc.vector.tensor_tensor(
                        out=att[0:D, nt, 0:289],
                        in0=pa[0:D, 0:289],
                        in1=rb[0:D, 0:289],
                        op=mybir.AluOpType.mult,
                    )
                    nc.vector.tensor_tensor(
                        out=att[0:D, nt, 288:577],
                        in0=pb[0:D, 0:289],
                        in1=rb[0:D, 288:577],
                        op=mybir.AluOpType.mult,
                    )
                else:
                    stg = stgp.tile([D, N], cdt, tag="stg", name=f"st_{next(_uid)}")
                    nc.vector.tensor_tensor(
                        out=stg[:, 0:289],
                        in0=pa[0:D, 0:289],
                        in1=rb[0:D, 0:289],
                        op=mybir.AluOpType.mult,
                    )
                    nc.vector.tensor_tensor(
                        out=stg[:, 288:577],
                        in0=pb[0:D, 0:289],
                        in1=rb[0:D, 288:577],
                        op=mybir.AluOpType.mult,
                    )
                    nc.sync.dma_start(out=att[D:128, nt, :], in_=stg[:])

            def emit_proj_unit(b, s, ti, att, wp):
                t0, tl = TOKT[ti]
                ps = psp.tile([128, 2, 512], F32, tag="sc", name=f"ps_{next(_uid)}")
                for ci in range(2):
                    for kk in range(KT):
                        nc.tensor.matmul(
                            ps[0:tl, ci, :],
                            lhsT=att[:, kk, t0:t0 + tl],
                            rhs=wp[:, kk, 4 * ci:4 * ci + 4, :],
                            start=(kk == 0),
                            stop=(kk == KT - 1),
                        )
                ob = outp.tile([128, C], F32, tag="ob", name=f"ob_{next(_uid)}")
                for ci in range(2):
                    nc.vector.tensor_tensor(
                        out=ob[0:tl, 512 * ci:512 * ci + 512],
                        in0=ps[0:tl, ci, :],
                        in1=bias_sb[0:tl, 512 * ci:512 * ci + 512],
                        op=mybir.AluOpType.add,
                    )
                nc.sync.dma_start(
                    out=out[2 * b + s, t0:t0 + tl, :],
                    in_=ob[0:tl, :],
                )

            loop_ctx = (
                tc.For_i(0, reps, 1) if reps > 1 else contextlib.nullcontext()
            )
            with loop_ctx:
                load_batch_inputs(0)
                for n in range(4):
                    emit_qk_unit(0, n)
                wv = w8vp.tile([128, KT, 8, 128], cdt, tag="wv",
                               name=f"wv_{next(_uid)}")
                nc.sync.dma_start(out=wv[:], in_=wvd[:])
                for n in range(4, 16):
                    emit_qk_unit(0, n)
                wp = None
                if mode not in ("qkv", "noproj"):
                    wp = w8pp.tile([128, KT, 8, 128], cdt, tag="wp",
                                   name=f"wp_{next(_uid)}")
                    nc.sync.dma_start(out=wp[:], in_=wproj[:])
                for s in range(2):
                    for ti in range(5):
                        emit_v_unit(0, s, ti, wv)

                for b in range(BL):
                    st = state[b]
                    feeders = []
                    if b + 1 < BL:
                        load_batch_inputs(b + 1)
                        feeders += [("qk", n) for n in range(16)]
                    fi = 0

                    att_sb = [None, None]
                    if mode != "qkv":
                        att_sb = []
                        for s in range(2):
                            att_sb.append(
                                xap.tile(
                                    [128, KT, N], cdt, tag="xa",
                                    name=f"att_{next(_uid)}",
                                )
                            )

                    proj_done = set()
                    for i, (s, h) in enumerate(
                        [(s, h) for s in range(2) for h in range(H)]
                    ):
                        if mode != "qkv":
                            emit_attn_head(b, s, h, att_sb[s])
                        if i % 2 == 0 and fi < len(feeders):
                            f = feeders[fi]
                            fi += 1
                            emit_qk_unit(b + 1, f[1])
                        elif (
                            mode == "full" and b + 1 >= BL and s == 1
                            and h >= H - 5
                        ):
                            ti = h - (H - 5)
                            emit_proj_unit(b, 0, ti, att_sb[0], wp)
                            proj_done.add((0, ti))
                    tail = (
                        [
                            ("p", s, ti)
                            for s in range(2)
                            for ti in range(5)
                            if (s, ti) not in proj_done
                        ]
                        if mode not in ("qkv", "noproj") else []
                    )
                    vunits = (
                        [("v", s, ti) for s in range(2) for ti in range(5)]
                        if b + 1 < BL else []
                    )
                    merged = []
                    for j in range(max(len(tail), len(vunits))):
                        if j < len(tail):
                            merged.append(tail[j])
                        if j < len(vunits):
                            merged.append(vunits[j])
                    for u in merged:
                        if u[0] == "p":
                            emit_proj_unit(b, u[1], u[2], att_sb[u[1]], wp)
                        else:
                            emit_v_unit(b + 1, u[1], u[2], wv)
                    del state[b]

    if split:
        split_excess_waits(nc)
    return nc


_CACHE = {}

CDT = mybir.dt.bfloat16  # compute dtype knob: bfloat16 | float32r | float32


def _get_nc(reps=1, mode="full"):
    key = (str(CDT), reps, mode)
    if key not in _CACHE:
        _CACHE[key] = build_kernel(CDT, reps=reps, mode=mode)
    return _CACHE[key]


def prep_in_maps(x1, x2, Wqkv, Wproj, bproj, cdt=None):
    cdt = cdt or CDT
    np_cdt = mybir.dt.np(cdt)
    x1 = np.asarray(x1, dtype=np.float32)
    x2 = np.asarray(x2, dtype=np.float32)
    Wqkv = np.asarray(Wqkv, dtype=np.float32)
    Wproj = np.asarray(Wproj, dtype=np.float32)
    bproj = np.asarray(bproj, dtype=np.float32)

    # wqk[n, p, kt, f] = Wqkv[kt*128+p, n*128+f]  (q,k output blocks)
    wqk = np.ascontiguousarray(
        Wqkv[:, : 2 * C].reshape(KT, 128, 16, 128).transpose(2, 1, 0, 3)
    ).astype(np_cdt)
    # wvd[p, kt, n, f] = Wqkv[kt*128+p, 2C + n*128+f]  (v output blocks)
    wvd = np.ascontiguousarray(
        Wqkv[:, 2 * C:].reshape(KT, 128, 8, 128).transpose(1, 0, 2, 3)
    ).astype(np_cdt)
    wp = np.ascontiguousarray(
        Wproj.reshape(KT, 128, 8, 128).transpose(1, 0, 2, 3)
    ).astype(np_cdt)
    biasr = np.ascontiguousarray(
        np.broadcast_to(bproj, (128, C))
    ).astype(np.float32)

    # [B, N, C] -> per-core [NSEQ, 128, KT, N]: xt[s, p, kt, n] = x[n, kt*128+p]
    xt_all = np.empty((NCORES, NSEQ, 128, KT, N), dtype=np_cdt)
    for c in range(NCORES):
        for lb in range(BL):
            b = BL * c + lb
            xt_all[c, 2 * lb + 0] = (
                x1[b].T.reshape(KT, 128, N).transpose(1, 0, 2).astype(np_cdt)
            )
            xt_all[c, 2 * lb + 1] = (
                x2[b].T.reshape(KT, 128, N).transpose(1, 0, 2).astype(np_cdt)
            )

    return [
        {"xt": xt_all[c], "wqk": wqk, "wvd": wvd, "wproj": wp, "biasr": biasr}
        for c in range(NCORES)
    ]


def unpack_results(results):
    out1 = np.empty((NCORES * BL, N, C), dtype=np.float32)
    out2 = np.empty((NCORES * BL, N, C), dtype=np.float32)
    for c in range(NCORES):
        o = results[c]["out"]
        for lb in range(BL):
            out1[BL * c + lb] = o[2 * lb + 0]
            out2[BL * c + lb] = o[2 * lb + 1]
    return out1, out2


def kernel(x1, x2, Wqkv, Wproj, bproj):
    nc = _get_nc()
    in_maps = prep_in_maps(x1, x2, Wqkv, Wproj, bproj)
    res = run_bass_kernel_spmd(nc, in_maps, core_ids=list(range(NCORES)))
    return unpack_results(res.results)


# revision 42
# speedup vs baseline: 1.6283x; 1.0231x over previous
"""Trainium2 Bass kernel for dual-stream cross/self attention (nn_Attention).

Reference semantics (per batch b):
  qkv_s = x_s @ Wqkv  -> q_s,k_s,v_s  [H=16 heads, N=577 tokens, d=64]
  stream s output head h attends with q_s and (k_s,v_s) if h<10 else (k_o,v_o)
  out_s = concat_heads @ Wproj + bproj

Sharding: batch (16) data-parallel over 8 cores, 2 batches/core; weights
replicated. Per core, 4 sequences (2 batches x 2 streams) are processed.

This is the baseline dataflow/emission order (which measures fastest on HW)
with only the DMA layer reworked:
  - host packs every input so each DMA lands per-partition contiguous
    chunks >= 512B (sub-512B chunks halve DMA bus efficiency)
  - one DMA per qk weight unit ([128, KT, 128]) instead of 8 tile loads
  - wv / wproj loaded once per rep (single DMA each) instead of per batch
  - one output DMA per proj token tile ([tl, 1024]) instead of two
This cuts the per-rep DMA count from ~557 to ~290 without touching the PE
instruction mix, the softmax normalization, or the head schedule.

Per-core dataflow (all matmuls contract over the SBUF partition dim):
  - q,k computed as [c_out, tok] (c-major); v as [tok, c_out] (tok-major),
    with a constant ones-column appended per head so the attention matmul
    also produces the softmax denominator for free
  - scores^T[j,i] = k_head^T q_head, exp via ScalarE (scale fused, no max
    subtraction -- |score*scale| <= ~3.5 for this problem so exp is safe)
  - attn^T[d,i] (+ sumexp row) = [v_head|1]^T @ exp^T, accumulated over j
  - normalize along tokens via reciprocal + DMA partition-broadcast + mult
  - proj: out[tok,c] = attn^T-tiles^T @ Wproj-tiles, bias added from a
    host-replicated [128,1024] bias tile
"""

import numpy as np

import concourse.bass as bass
import concourse.mybir as mybir
import concourse.tile as tile
from concourse.bass_utils import run_bass_kernel_spmd

# ---------------------------------------------------------------------------
# Workaround: this walrus build rejects any instruction carrying >1 sem wait
# ("Too many sync wait commands").  Post-process the scheduled program and
# move excess waits onto single-wait NoOps inserted just before, on the same
# engine (engines execute their stream in order, so this is equivalent).
# ---------------------------------------------------------------------------


def split_excess_waits(nc, max_waits=1):
    cnt = 0
    for f in nc.m.functions:
        for blk in f.blocks:
            insts = blk.instructions
            need = any(
                inst.sync_info is not None
                and len(inst.sync_info.on_wait) > max_waits
                for inst in insts
            )
            if not need:
                continue
            newl = []
            for inst in insts:
                si = inst.sync_info
                if si is not None and len(si.on_wait) > max_waits:
                    waits = list(si.on_wait)
                    for w in waits[max_waits:]:
                        nop = mybir.InstNoOp(
                            name=f"wsplit_{cnt}",
                            engine=inst.engine,
                            ins=[],
                            outs=[],
                            sync_info=mybir.SyncInfo(on_wait=[w], on_update=[]),
                        )
                        cnt += 1
                        newl.append(nop)
                    si.on_wait = waits[:max_waits]
                newl.append(inst)
            blk.instructions = newl
    return cnt

# ---------------------------------------------------------------------------

F32 = mybir.dt.float32

N = 577          # tokens
C = 1024         # model dim
H = 16           # heads
D = 64           # head dim
HS = 10          # first HS heads self-attend, rest cross-attend
KT = 8           # c_in tiles of 128
SCALE = D ** -0.5
NCORES = 8
BL = 2           # local batches per core
NSEQ = 2 * BL    # sequences per core (batch-major, stream-minor)

# token partition tiles (start, len)
TOKT = [(0, 128), (128, 128), (256, 128), (384, 128), (512, 65)]
# token free-dim chunks (start, len): overlap 1 col at 288 so both are 289
# wide and a single ScalarE op can cover both PSUM sub-banks garbage-free
CH = [(0, 289), (288, 289)]

def build_kernel(cdt, reps=1, mode="full", split=True):
    nc = bass.Bass()
    xt = nc.dram_tensor("xt", [NSEQ, 128, KT, N], cdt, kind="ExternalInput")
    wqk = nc.dram_tensor("wqk", [16, 128, KT, 128], cdt, kind="ExternalInput")
    wvd = nc.dram_tensor("wvd", [128, KT, 8, 128], cdt, kind="ExternalInput")
    wproj = nc.dram_tensor("wproj", [128, KT, 8, 128], cdt, kind="ExternalInput")
    biasr = nc.dram_tensor("biasr", [128, C], F32, kind="ExternalInput")
    out = nc.dram_tensor("out", [NSEQ, N, C], F32, kind="ExternalOutput")

    import contextlib
    import itertools
    _uid = itertools.count()

    with tile.TileContext(nc) as tc:
        with (
            tc.tile_pool(name="const", bufs=1) as constp,
            tc.tile_pool(name="xa", bufs=4) as xap,       # xt + attnT share
            tc.tile_pool(name="qk", bufs=8) as qkp,       # q,k of 2 batches
            tc.tile_pool(name="vp", bufs=2) as vpp,
            tc.tile_pool(name="ep", bufs=3) as epp,
            tc.tile_pool(name="w1", bufs=3) as w1p,       # streamed qk weights
            tc.tile_pool(name="w8v", bufs=1) as w8vp,     # wv tile
            tc.tile_pool(name="w8p", bufs=1) as w8pp,     # wproj tile
            tc.tile_pool(name="rbp", bufs=4) as rbpp,     # recip + broadcast
            tc.tile_pool(name="stg", bufs=2) as stgp,     # odd-head staging
            tc.tile_pool(name="op", bufs=2) as outp,
            tc.tile_pool(name="dr", bufs=4, space="DRAM") as drp,
            tc.tile_pool(name="ps", bufs=2, space="PSUM") as psp,
        ):
            bias_sb = constp.tile([128, C], F32, tag="bias")
            nc.sync.dma_start(out=bias_sb[:], in_=biasr[:])

            state = {}

            def load_batch_inputs(b):
                st = {}
                st["xts"] = []
                for s in range(2):
                    t = xap.tile([128, KT, N], cdt, tag="xa", name=f"xt_{next(_uid)}")
                    nc.sync.dma_start(out=t[:], in_=xt[2 * b + s])
                    st["xts"].append(t)
                st["q"] = [
                    qkp.tile([128, 8, N], cdt, tag="qk", name=f"q_{next(_uid)}")
                    for s in range(2)
                ]
                st["k"] = [
                    qkp.tile([128, 8, N], cdt, tag="qk", name=f"k_{next(_uid)}")
                    for s in range(2)
                ]
                st["v"] = None
                state[b] = st

            def ensure_v(b):
                st = state[b]
                if st["v"] is None:
                    st["v"] = []
                    for s in range(2):
                        v = vpp.tile(
                            [128, 5, H, D + 1], cdt, tag="v",
                            name=f"v_{next(_uid)}",
                        )
                        nc.vector.memset(v[:, :, :, D:D + 1], 1.0)
                        st["v"].append(v)

            def emit_qk_unit(b, n):
                st = state[b]
                w = w1p.tile([128, KT, 128], cdt, tag="w1", name=f"wqk_{next(_uid)}")
                nc.sync.dma_start(out=w[:], in_=wqk[n])
                for s in range(2):
                    dst = st["q"][s] if n < 8 else st["k"][s]
                    nd = n % 8
                    ps = psp.tile([128, 2, 512], F32, tag="sc", name=f"ps_{next(_uid)}")
                    for ci, (c0, cl) in enumerate(CH):
                        for kk in range(KT):
                            nc.tensor.matmul(
                                ps[:, ci, 0:cl],
                                lhsT=w[:, kk, :],
                                rhs=st["xts"][s][:, kk, c0:c0 + cl],
                                start=(kk == 0),
                                stop=(kk == KT - 1),
                            )
                    nc.vector.tensor_copy(out=dst[:, nd, 0:289], in_=ps[:, 0, 0:289])
                    nc.vector.tensor_copy(out=dst[:, nd, 288:577], in_=ps[:, 1, 0:289])

            def emit_v_unit(b, s, ti, wv):
                ensure_v(b)
                st = state[b]
                t0, tl = TOKT[ti]
                ps = psp.tile([128, 2, 512], F32, tag="sc", name=f"ps_{next(_uid)}")
                for ci in range(2):
                    for kk in range(KT):
                        nc.tensor.matmul(
                            ps[0:tl, ci, :],
                            lhsT=st["xts"][s][:, kk, t0:t0 + tl],
                            rhs=wv[:, kk, 4 * ci:4 * ci + 4, :],
                            start=(kk == 0),
                            stop=(kk == KT - 1),
                        )
                for ci in range(2):
                    nc.vector.tensor_copy(
                        out=st["v"][s][0:tl, ti, 8 * ci:8 * ci + 8, 0:D],
                        in_=ps[0:tl, ci, :].rearrange("p (h d) -> p h d", d=D),
                    )

            def emit_attn_head(b, s, h, att):
                st = state[b]
                ensure_v(b)
                kv = s if h < HS else 1 - s
                par = (h % 2) * D
                nt = h // 2

                et = epp.tile(
                    [128, 5, 2, 289], cdt, tag="et", name=f"et_{next(_uid)}"
                )
                for jt, (j0, jl) in enumerate(TOKT):
                    ps = psp.tile(
                        [128, 2, 512], F32, tag="sc", name=f"ps_{next(_uid)}"
                    )
                    for ci, (c0, cl) in enumerate(CH):
                        nc.tensor.matmul(
                            ps[0:jl, ci, 0:cl],
                            lhsT=st["k"][kv][par:par + D, nt, j0:j0 + jl],
                            rhs=st["q"][s][par:par + D, nt, c0:c0 + cl],
                            start=True,
                            stop=True,
                        )
                    nc.scalar.activation(
                        out=et[0:jl, jt],
                        in_=ps[0:jl, :, 0:289],
                        func=mybir.ActivationFunctionType.Exp,
                        scale=SCALE,
                    )

                pa = psp.tile([128, 512], F32, tag="paA", name=f"pa_{next(_uid)}")
                pb = psp.tile([128, 512], F32, tag="paB", name=f"pb_{next(_uid)}")
                for jt, (j0, jl) in enumerate(TOKT):
                    nc.tensor.matmul(
                        pa[0:D + 1, 0:289],
                        lhsT=st["v"][kv][0:jl, jt, h, :],
                        rhs=et[0:jl, jt, 0],
                        start=(jt == 0),
                        stop=(jt == 4),
                    )
                    nc.tensor.matmul(
                        pb[0:D + 1, 0:289],
                        lhsT=st["v"][kv][0:jl, jt, h, :],
                        rhs=et[0:jl, jt, 1],
                        start=(jt == 0),
                        stop=(jt == 4),
                    )

                rb = rbpp.tile([128, N], F32, tag="rb", name=f"rb_{next(_uid)}")
                nc.vector.reciprocal(
                    out=rb[D:D + 1, 0:289], in_=pa[D:D + 1, 0:289]
                )
                nc.vector.reciprocal(
                    out=rb[D:D + 1, 288:577], in_=pb[D:D + 1, 0:289]
                )
                rd = drp.tile([N], F32, tag="rd", name=f"rd_{next(_uid)}")
                nc.sync.dma_start(out=rd[None, :], in_=rb[D:D + 1, :])
                nc.sync.dma_start(
                    out=rb[0:D, 0:577],
                    in_=rd[None, 0:577].to_broadcast([D, 577]),
                )
                if par == 0:
                    nc.vector.tensor_tensor(
                        out=att[0:D, nt, 0:289],
                        in0=pa[0:D, 0:289],
                        in1=rb[0:D, 0:289],
                        op=mybir.AluOpType.mult,
                    )
                    nc.vector.tensor_tensor(
                        out=att[0:D, nt, 288:577],
                        in0=pb[0:D, 0:289],
                        in1=rb[0:D, 288:577],
                        op=mybir.AluOpType.mult,
                    )
                else:
                    stg = stgp.tile([D, N], cdt, tag="stg", name=f"st_{next(_uid)}")
                    nc.vector.tensor_tensor(
                        out=stg[:, 0:289],
                        in0=pa[0:D, 0:289],
                        in1=rb[0:D, 0:289],
                        op=mybir.AluOpType.mult,
                    )
                    nc.vector.tensor_tensor(
                        out=stg[:, 288:577],
                        in0=pb[0:D, 0:289],
                        in1=rb[0:D, 288:577],
                        op=mybir.AluOpType.mult,
                    )
                    nc.sync.dma_start(out=att[D:128, nt, :], in_=stg[:])

            def emit_proj_unit(b, s, ti, att, wp):
                t0, tl = TOKT[ti]
                ps = psp.tile([128, 2, 512], F32, tag="sc", name=f"ps_{next(_uid)}")
                for ci in range(2):
                    for kk in range(KT):
                        nc.tensor.matmul(
                            ps[0:tl, ci, :],
                            lhsT=att[:, kk, t0:t0 + tl],
                            rhs=wp[:, kk, 4 * ci:4 * ci + 4, :],
                            start=(kk == 0),
                            stop=(kk == KT - 1),
                        )
                ob = outp.tile([128, C], F32, tag="ob", name=f"ob_{next(_uid)}")
                for ci in range(2):
                    nc.vector.tensor_tensor(
                        out=ob[0:tl, 512 * ci:512 * ci + 512],
                        in0=ps[0:tl, ci, :],
                        in1=bias_sb[0:tl, 512 * ci:512 * ci + 512],
                        op=mybir.AluOpType.add,
                    )
                nc.sync.dma_start(
                    out=out[2 * b + s, t0:t0 + tl, :],
                    in_=ob[0:tl, :],
                )

            loop_ctx = (
                tc.For_i(0, reps, 1) if reps > 1 else contextlib.nullcontext()
            )
            with loop_ctx:
                load_batch_inputs(0)
                for n in range(4):
                    emit_qk_unit(0, n)
                wv = w8vp.tile([128, KT, 8, 128], cdt, tag="wv",
                               name=f"wv_{next(_uid)}")
                nc.sync.dma_start(out=wv[:], in_=wvd[:])
                for n in range(4, 16):
                    emit_qk_unit(0, n)
                wp = None
                if mode not in ("qkv", "noproj"):
                    wp = w8pp.tile([128, KT, 8, 128], cdt, tag="wp",
                                   name=f"wp_{next(_uid)}")
                    nc.sync.dma_start(out=wp[:], in_=wproj[:])
                for s in range(2):
                    for ti in range(5):
                        emit_v_unit(0, s, ti, wv)

                for b in range(BL):
                    st = state[b]
                    feeders = []
                    if b + 1 < BL:
                        load_batch_inputs(b + 1)
                        feeders += [("qk", n) for n in range(16)]
                    fi = 0

                    att_sb = [None, None]
                    if mode != "qkv":
                        att_sb = []
                        for s in range(2):
                            att_sb.append(
                                xap.tile(
                                    [128, KT, N], cdt, tag="xa",
                                    name=f"att_{next(_uid)}",
                                )
                            )

                    proj_done = set()
                    for i, (s, h) in enumerate(
                        [(s, h) for s in range(2) for h in range(H)]
                    ):
                        if mode != "qkv":
                            emit_attn_head(b, s, h, att_sb[s])
                        if i % 2 == 0 and fi < len(feeders):
                            f = feeders[fi]
                            fi += 1
                            emit_qk_unit(b + 1, f[1])
                        elif (
                            mode == "full" and b + 1 >= BL and s == 1
                            and h >= H - 5
                        ):
                            ti = h - (H - 5)
                            emit_proj_unit(b, 0, ti, att_sb[0], wp)
                            proj_done.add((0, ti))
                    tail = (
                        [
                            ("p", s, ti)
                            for s in range(2)
                            for ti in range(5)
                            if (s, ti) not in proj_done
                        ]
                        if mode not in ("qkv", "noproj") else []
                    )
                    vunits = (
                        [("v", s, ti) for s in range(2) for ti in range(5)]
                        if b + 1 < BL else []
                    )
                    merged = []
                    for j in range(max(len(tail), len(vunits))):
                        if j < len(tail):
                            merged.append(tail[j])
                        if j < len(vunits):
                            merged.append(vunits[j])
                    for u in merged:
                        if u[0] == "p":
                            emit_proj_unit(b, u[1], u[2], att_sb[u[1]], wp)
                        else:
                            emit_v_unit(b + 1, u[1], u[2], wv)
                    del state[b]

    if split:
        split_excess_waits(nc)
    return nc


_CACHE = {}

CDT = mybir.dt.bfloat16  # compute dtype knob: bfloat16 | float32r | float32


def _get_nc(reps=1, mode="full"):
    key = (str(CDT), reps, mode)
    if key not in _CACHE:
        _CACHE[key] = build_kernel(CDT, reps=reps, mode=mode)
    return _CACHE[key]


def prep_in_maps(x1, x2, Wqkv, Wproj, bproj, cdt=None):
    cdt = cdt or CDT
    np_cdt = mybir.dt.np(cdt)
    x1 = np.asarray(x1, dtype=np.float32)
    x2 = np.asarray(x2, dtype=np.float32)
    Wqkv = np.asarray(Wqkv, dtype=np.float32)
    Wproj = np.asarray(Wproj, dtype=np.float32)
    bproj = np.asarray(bproj, dtype=np.float32)

    # wqk[n, p, kt, f] = Wqkv[kt*128+p, n*128+f]  (q,k output blocks)
    wqk = np.ascontiguousarray(
        Wqkv[:, : 2 * C].reshape(KT, 128, 16, 128).transpose(2, 1, 0, 3)
    ).astype(np_cdt)
    # wvd[p, kt, n, f] = Wqkv[kt*128+p, 2C + n*128+f]  (v output blocks)
    wvd = np.ascontiguousarray(
        Wqkv[:, 2 * C:].reshape(KT, 128, 8, 128).transpose(1, 0, 2, 3)
    ).astype(np_cdt)
    wp = np.ascontiguousarray(
        Wproj.reshape(KT, 128, 8, 128).transpose(1, 0, 2, 3)
    ).astype(np_cdt)
    biasr = np.ascontiguousarray(
        np.broadcast_to(bproj, (128, C))
    ).astype(np.float32)

    # [B, N, C] -> per-core [NSEQ, 128, KT, N]: xt[s, p, kt, n] = x[n, kt*128+p]
    xt_all = np.empty((NCORES, NSEQ, 128, KT, N), dtype=np_cdt)
    for c in range(NCORES):
        for lb in range(BL):
            b = BL * c + lb
            xt_all[c, 2 * lb + 0] = (
                x1[b].T.reshape(KT, 128, N).transpose(1, 0, 2).astype(np_cdt)
            )
            xt_all[c, 2 * lb + 1] = (
                x2[b].T.reshape(KT, 128, N).transpose(1, 0, 2).astype(np_cdt)
            )

    return [
        {"xt": xt_all[c], "wqk": wqk, "wvd": wvd, "wproj": wp, "biasr": biasr}
        for c in range(NCORES)
    ]


def unpack_results(results):
    out1 = np.empty((NCORES * BL, N, C), dtype=np.float32)
    out2 = np.empty((NCORES * BL, N, C), dtype=np.float32)
    for c in range(NCORES):
        o = results[c]["out"]
        for lb in range(BL):
            out1[BL * c + lb] = o[2 * lb + 0]
            out2[BL * c + lb] = o[2 * lb + 1]
    return out1, out2


def kernel(x1, x2, Wqkv, Wproj, bproj):
    nc = _get_nc()
    in_maps = prep_in_maps(x1, x2, Wqkv, Wproj, bproj)
    res = run_bass_kernel_spmd(nc, in_maps, core_ids=list(range(NCORES)))
    return unpack_results(res.results)
